# revision 22
# baseline (speedup 1.0000x reference)
"""Expert-choice MoE (8 experts, shared SwiGLU weights) on 8 trn2 NeuronCores.

Sharding: expert-parallel, one expert per core. Routing (tiny: 8192x2048x8
matmul + softmax + top-k) runs on host CPU with the exact same jax ops as the
reference so the discrete top-k selection is bit-identical; scatter-add is
order-invariant so only the selected set matters. The heavy FFN
(~69 GFLOP/core) runs on device in bf16 with fp32 PSUM accumulation.

Device kernel per core (expert e):
  phase B: hT[dff, tok] = silu(Wg^T x + bg) * (Wu^T x + bu)   (dff on partitions)
  phase C: y[tok, H]    = (hT^T @ Wd + b_down) * G[:, e]      (tok on partitions)
All operands are packed host-side so every matmul reads natural K-major SBUF
slices; no on-device transposes. b_down enters as a rank-1 (ones x b_down)
matmul accumulated into the same PSUM group, split hi/lo in bf16 for accuracy.
"""

import os
import sys

for _p in ("/opt/trn_rl_repo",):
    if _p not in sys.path and os.path.isdir(_p):
        sys.path.insert(0, _p)

import numpy as np
from contextlib import ExitStack

import jax
import jax.numpy as jnp

import concourse.bass as bass
import concourse.tile as tile
from concourse import bacc, mybir
from concourse import bass_utils as _bass_utils
from concourse.bass_utils import run_bass_kernel_spmd


def _ensure_axon_hooks():
    """Provide antenv.axon_hooks if the image lacks it, so trace=True /
    BASS_TRACE profiling works (and degrades gracefully when it can't)."""
    try:
        import antenv.axon_hooks  # noqa: F401

        return
    except ImportError:
        pass
    import types
    import ctypes
    import contextlib

    mod = types.ModuleType("antenv.axon_hooks")
    _hook_box = [None]

    def set_axon_ntff_profile_hook(h):
        _hook_box[0] = h

    def get_axon_ntff_profile_hook():
        return _hook_box[0]

    mod.set_axon_ntff_profile_hook = set_axon_ntff_profile_hook
    mod.get_axon_ntff_profile_hook = get_axon_ntff_profile_hook

    so_path = "/opt/axon/libaxon_pjrt.so"
    if os.path.exists(so_path):
        try:
            lib = ctypes.CDLL(so_path)
            if hasattr(lib, "axon_start_nrt_profile"):
                lib.axon_start_nrt_profile.argtypes = [
                    ctypes.POINTER(ctypes.c_int64),
                    ctypes.c_size_t,
                ]
                lib.axon_start_nrt_profile.restype = ctypes.c_int64
                lib.axon_stop_nrt_profile.argtypes = [ctypes.c_char_p]
                lib.axon_stop_nrt_profile.restype = ctypes.c_int64

                @contextlib.contextmanager
                def _hook(output_dir, device_ids):
                    import jax as _jax

                    _jax.devices()
                    if device_ids:
                        ids = (ctypes.c_int64 * len(device_ids))(*device_ids)
                        rc = lib.axon_start_nrt_profile(ids, len(device_ids))
                    else:
                        rc = lib.axon_start_nrt_profile(None, 0)
                    if rc != 0:
                        raise RuntimeError(f"axon_start_nrt_profile rc={rc}")
                    try:
                        yield
                    finally:
                        n = lib.axon_stop_nrt_profile(str(output_dir).encode())
                        print(f"profile: {n} file(s) written to {output_dir}", file=sys.stderr)

                _hook_box[0] = _hook
        except OSError:
            pass

    import antenv

    sys.modules["antenv.axon_hooks"] = mod
    antenv.axon_hooks = mod


_ensure_axon_hooks()

# upload_artifacts reaches for a remote bucket; in this sandbox just keep
# artifacts local (only used on the trace path).
_orig_upload = _bass_utils.upload_artifacts


def _safe_upload(tmpdir):
    try:
        return _orig_upload(tmpdir)
    except Exception:
        return str(tmpdir)


_bass_utils.upload_artifacts = _safe_upload

# ---------------------------------------------------------------- constants
E = 8            # experts == cores
H = 2048         # hidden
DFF = 5461       # ffn inner
NDT = 43         # dff tiles of 128 (padded)
DFFP = NDT * 128 # 5504
KTOK = 1024      # expert capacity (top-k per expert)
NTOK = 8192      # total tokens
NKC = H // 128   # 16 K-chunks for gate/up
NTC = 2          # token chunks of 512 (gate/up free dim)
NTT = 8          # token tiles of 128 (down partition dim)
NHC = 4          # H chunks of 512 (down free dim)
EPS = 1e-10

BF16 = mybir.dt.bfloat16
F32 = mybir.dt.float32

# ---------------------------------------------------------------- device IR
def _ffn_body(ctx, tc, xt, wg, wu, wd, bg, bu, bd, gv, y):
    nc = tc.nc

    const_pool = ctx.enter_context(tc.tile_pool(name="const", bufs=1))
    h_pool = ctx.enter_context(tc.tile_pool(name="h", bufs=1))
    out_pool = ctx.enter_context(tc.tile_pool(name="out", bufs=2))
    psum_yp = ctx.enter_context(tc.tile_pool(name="psum_y", bufs=4, space="PSUM"))

    # constants / small tensors
    bg_sb = const_pool.tile([128, NDT], F32)
    nc.gpsimd.dma_start(bg_sb[:], bg[:])
    bu_sb = const_pool.tile([128, NDT], F32)
    nc.gpsimd.dma_start(bu_sb[:], bu[:])
    bd_hi = const_pool.tile([1, H], BF16)
    nc.gpsimd.dma_start(bd_hi[:], bd[0:1, :])
    gv_sb = const_pool.tile([128, NTT], F32)
    nc.gpsimd.dma_start(gv_sb[:], gv[:])
    ones_sb = const_pool.tile([1, 128], BF16)
    nc.vector.memset(ones_sb[:], 1.0)

    # hT[dff, tok] as 43 tiles of [128, 1024] packed along free dim
    h_all = h_pool.tile([128, NDT * KTOK], BF16)

    # wd pool opened early so hc=0 slab loads can overlap late phase B
    wd_pool = ctx.enter_context(tc.tile_pool(name="wdp", bufs=48))
    wd_sbs = {}

    def _load_wd(hc, dc):
        wds = wd_pool.tile([128, 512], BF16, tag="wds")
        nc.sync.dma_start(wds[:], wd[:, hc, dc * 512 : (dc + 1) * 512])
        wd_sbs[(hc, dc)] = wds

    # ---------------- phase B: gate/up + silu*up -> h_all ----------------
    with (
        tc.tile_pool(name="xt", bufs=1) as xt_pool,
        tc.tile_pool(name="guw", bufs=3) as guw_pool,
        tc.tile_pool(name="gus", bufs=2) as gus_pool,
        tc.tile_pool(name="psum_gu", bufs=2, space="PSUM") as psum_gu,
    ):
        xt_sb = xt_pool.tile([128, NKC * KTOK], BF16)
        for kc in range(NKC):
            nc.sync.dma_start(
                xt_sb[:, kc * KTOK : (kc + 1) * KTOK], xt[:, kc * KTOK : (kc + 1) * KTOK]
            )

        for ti in range(NDT):
            wg_sb = guw_pool.tile([128, H], BF16, tag="wgslab")
            nc.gpsimd.dma_start(wg_sb[:], wg[:, ti, :])
            wu_sb = guw_pool.tile([128, H], BF16, tag="wuslab")
            nc.gpsimd.dma_start(wu_sb[:], wu[:, ti, :])
            if ti >= 22 and (ti - 22) * 2 < NDT:
                _load_wd(0, (ti - 22) * 2)
                if (ti - 22) * 2 + 1 < NDT:
                    _load_wd(0, (ti - 22) * 2 + 1)
            for tcx in range(NTC):
                pg = psum_gu.tile([128, 512], F32, tag="pg")
                pu = psum_gu.tile([128, 512], F32, tag="pu")
                for kc in range(NKC):
                    rhs = xt_sb[:, kc * KTOK + tcx * 512 : kc * KTOK + tcx * 512 + 512]
                    nc.tensor.matmul(
                        pg[:],
                        wg_sb[:, kc * 128 : (kc + 1) * 128],
                        rhs,
                        start=(kc == 0),
                        stop=(kc == NKC - 1),
                    )
                for kc in range(NKC):
                    rhs = xt_sb[:, kc * KTOK + tcx * 512 : kc * KTOK + tcx * 512 + 512]
                    nc.tensor.matmul(
                        pu[:],
                        wu_sb[:, kc * 128 : (kc + 1) * 128],
                        rhs,
                        start=(kc == 0),
                        stop=(kc == NKC - 1),
                    )
                gact = gus_pool.tile([128, 512], F32, tag="gact")
                nc.scalar.activation(
                    gact[:],
                    pg[:],
                    mybir.ActivationFunctionType.Silu,
                    bias=bg_sb[:, ti : ti + 1],
                )
                uact = gus_pool.tile([128, 512], F32, tag="uact")
                nc.scalar.activation(
                    uact[:],
                    pu[:],
                    mybir.ActivationFunctionType.Identity,
                    bias=bu_sb[:, ti : ti + 1],
                )
                hsl = h_all[:, ti * KTOK + tcx * 512 : ti * KTOK + tcx * 512 + 512]
                nc.vector.tensor_mul(hsl, gact[:], uact[:])

    # ---------------- phase C: y = (hT^T @ Wd + bd) * G ----------------
    for hc in range(NHC):
        for dc in range(NDT):
            if (hc, dc) not in wd_sbs:
                _load_wd(hc, dc)
        for tt in range(NTT):
            py = psum_yp.tile([128, 512], F32, tag="py")
            for dc in range(NDT):
                lhsT = h_all[:, dc * KTOK + tt * 128 : dc * KTOK + tt * 128 + 128]
                nc.tensor.matmul(
                    py[:], lhsT, wd_sbs[(hc, dc)][:], start=(dc == 0), stop=False
                )
            # += ones^T @ b_down (bf16), closes the group
            nc.tensor.matmul(
                py[:], ones_sb[:], bd_hi[:, hc * 512 : (hc + 1) * 512],
                start=False, stop=True,
            )
            ot = out_pool.tile([128, 512], F32, tag="ot")
            nc.scalar.activation(
                ot[:],
                py[:],
                mybir.ActivationFunctionType.Copy,
                scale=gv_sb[:, tt : tt + 1],
            )
            nc.scalar.dma_start(
                y[tt * 128 : (tt + 1) * 128, hc * 512 : (hc + 1) * 512], ot[:]
            )


_NC_CACHE = {}


def _get_program():
    if "nc" not in _NC_CACHE:
        nc = bacc.Bacc("TRN2", target_bir_lowering=False, debug=False)
        aps = {
            "xt": nc.dram_tensor("xt", [128, NKC * KTOK], BF16, kind="ExternalInput").ap(),
            "wg": nc.dram_tensor("wg", [128, NDT, H], BF16, kind="ExternalInput").ap(),
            "wu": nc.dram_tensor("wu", [128, NDT, H], BF16, kind="ExternalInput").ap(),
            "wd": nc.dram_tensor("wd", [128, NHC, NDT * 512], BF16, kind="ExternalInput").ap(),
            "bg": nc.dram_tensor("bg", [128, NDT], F32, kind="ExternalInput").ap(),
            "bu": nc.dram_tensor("bu", [128, NDT], F32, kind="ExternalInput").ap(),
            "bd": nc.dram_tensor("bd", [2, H], BF16, kind="ExternalInput").ap(),
            "gv": nc.dram_tensor("gv", [128, NTT], F32, kind="ExternalInput").ap(),
            "y": nc.dram_tensor("y", [KTOK, H], F32, kind="ExternalOutput").ap(),
        }
        with tile.TileContext(nc) as tc, ExitStack() as ctx:
            _ffn_body(ctx, tc, **aps)
        nc.compile()
        _NC_CACHE["nc"] = nc
    return _NC_CACHE["nc"]


# ---------------------------------------------------------------- host side
_CPU = jax.devices("cpu")[0]


def _route(xf, router_w, router_b):
    """Exact replica of the reference routing, eagerly on CPU so the top-k
    index selection is bit-identical to the reference."""
    with jax.default_device(_CPU):
        xf = jax.device_put(jnp.asarray(xf), _CPU)
        clean_h = xf @ jax.device_put(jnp.asarray(router_w), _CPU) + jax.device_put(
            jnp.asarray(router_b), _CPU
        )
        u = jax.random.uniform(jax.random.key(42), clean_h.shape, dtype=clean_h.dtype)
        gumbel = -jnp.log(-jnp.log(u + EPS) + EPS)
        h = clean_h + gumbel  # ROUTER_TEMP = 1.0
        Sm = jax.nn.softmax(h, axis=-1)
        Gt, idxt = jax.lax.top_k(Sm.T, KTOK)  # [E, k]
        return np.asarray(Gt), np.asarray(idxt)


def _pack_weights_np(w_gate, w_up, w_down, b_gate, b_up, b_down):
    bf16 = jnp.bfloat16.dtype

    wg_p = np.zeros((H, DFFP), np.float32)
    wg_p[:, :DFF] = w_gate
    wu_p = np.zeros((H, DFFP), np.float32)
    wu_p[:, :DFF] = w_up
    wd_p = np.zeros((DFFP, H), np.float32)
    wd_p[:DFF, :] = w_down

    # [H, DFFP] -> [p, ti, kc*128+m]
    wg_dev = np.ascontiguousarray(
        wg_p.reshape(NKC, 128, NDT, 128).transpose(1, 2, 0, 3).reshape(128, NDT, H)
    ).astype(bf16)
    wu_dev = np.ascontiguousarray(
        wu_p.reshape(NKC, 128, NDT, 128).transpose(1, 2, 0, 3).reshape(128, NDT, H)
    ).astype(bf16)
    # [DFFP, H] -> [p, hc, dc*512+m]
    wd_dev = np.ascontiguousarray(
        wd_p.reshape(NDT, 128, NHC, 512).transpose(1, 2, 0, 3).reshape(128, NHC, NDT * 512)
    ).astype(bf16)

    bgp = np.zeros((DFFP,), np.float32)
    bgp[:DFF] = b_gate
    bup = np.zeros((DFFP,), np.float32)
    bup[:DFF] = b_up
    bg_dev = np.ascontiguousarray(bgp.reshape(NDT, 128).T)
    bu_dev = np.ascontiguousarray(bup.reshape(NDT, 128).T)

    bd_hi = b_down.astype(bf16)
    bd_lo = (b_down - bd_hi.astype(np.float32)).astype(bf16)
    bd_dev = np.ascontiguousarray(np.stack([bd_hi, bd_lo], axis=0))  # [2, H]
    return wg_dev, wu_dev, wd_dev, bg_dev, bu_dev, bd_dev


def _gather_pack_fn(xf, idxt):
    # toks[e] = xf[idxt[e]]  -> xt[e][p, kc*1024+t] = toks[e][t, kc*128+p]
    toks = jnp.take(xf, idxt.reshape(-1), axis=0).reshape(E, KTOK, H)
    xt = toks.reshape(E, KTOK, NKC, 128).transpose(0, 3, 2, 1).reshape(E, 128, NKC * KTOK)
    return xt.astype(jnp.bfloat16)


def _scatter_out_fn(dev_outs, idxt):
    # dev_outs: [E, KTOK, H] (already G-weighted, b_down included)
    yf = jnp.zeros((NTOK, H), jnp.float32)
    yf = yf.at[idxt.reshape(-1)].add(dev_outs.reshape(-1, H))
    return yf


_gather_pack = jax.jit(_gather_pack_fn)
_scatter_out = jax.jit(_scatter_out_fn)

LAST_RESULTS = None


def kernel(x, router_w, router_b, w_gate, b_gate, w_up, b_up, w_down, b_down):
    global LAST_RESULTS
    B, S, _ = x.shape
    xf_np = np.asarray(x, dtype=np.float32).reshape(NTOK, H)

    Gt, idxt = _route(xf_np, np.asarray(router_w), np.asarray(router_b))

    with jax.default_device(_CPU):
        xt_all = np.asarray(
            _gather_pack(jax.device_put(xf_np, _CPU), jax.device_put(idxt, _CPU))
        )

    wg_dev, wu_dev, wd_dev, bg_dev, bu_dev, bd_dev = _pack_weights_np(
        np.asarray(w_gate, np.float32),
        np.asarray(w_up, np.float32),
        np.asarray(w_down, np.float32),
        np.asarray(b_gate, np.float32),
        np.asarray(b_up, np.float32),
        np.asarray(b_down, np.float32),
    )

    # G columns per expert: gv[p, tt] = G[tt*128+p] for expert e; Gt is [E, k]
    gv_all = np.ascontiguousarray(
        Gt.reshape(E, NTT, 128).transpose(0, 2, 1).astype(np.float32)
    )

    nc = _get_program()
    in_maps = []
    for e in range(E):
        in_maps.append(
            {
                "xt": np.ascontiguousarray(xt_all[e]),
                "wg": wg_dev,
                "wu": wu_dev,
                "wd": wd_dev,
                "bg": bg_dev,
                "bu": bu_dev,
                "bd": bd_dev,
                "gv": gv_all[e],
            }
        )
    res = run_bass_kernel_spmd(nc, in_maps, list(range(E)))
    LAST_RESULTS = res

    dev_outs = np.stack([res.results[e]["y"] for e in range(E)], axis=0)

    with jax.default_device(_CPU):
        yf = np.asarray(
            _scatter_out(jax.device_put(dev_outs, _CPU), jax.device_put(idxt, _CPU))
        )
    return yf.reshape(B, S, H)


# revision 23
# speedup vs baseline: 1.0077x; 1.0077x over previous
"""Expert-choice MoE (8 experts, shared SwiGLU weights) on 8 trn2 NeuronCores.

Sharding: expert-parallel, one expert per core. Routing (tiny: 8192x2048x8
matmul + softmax + top-k) runs on host CPU with the exact same jax ops as the
reference so the discrete top-k selection is bit-identical; scatter-add is
order-invariant so only the selected set matters. The heavy FFN
(~69 GFLOP/core) runs on device in bf16 with fp32 PSUM accumulation.

Device kernel per core (expert e):
  phase B: hT[dff, tok] = silu(Wg^T x + bg) * (Wu^T x + bu)   (dff on partitions)
  phase C: y[tok, H]    = (hT^T @ Wd + b_down) * G[:, e]      (tok on partitions)
All operands are packed host-side so every matmul reads natural K-major SBUF
slices; no on-device transposes. b_down enters as a rank-1 (ones x b_down)
matmul accumulated into the same PSUM group, split hi/lo in bf16 for accuracy.
"""

import os
import sys

for _p in ("/opt/trn_rl_repo",):
    if _p not in sys.path and os.path.isdir(_p):
        sys.path.insert(0, _p)

import numpy as np
from contextlib import ExitStack

import jax
import jax.numpy as jnp

import concourse.bass as bass
import concourse.tile as tile
from concourse import bacc, mybir
from concourse import bass_utils as _bass_utils
from concourse.bass_utils import run_bass_kernel_spmd


def _ensure_axon_hooks():
    """Provide antenv.axon_hooks if the image lacks it, so trace=True /
    BASS_TRACE profiling works (and degrades gracefully when it can't)."""
    try:
        import antenv.axon_hooks  # noqa: F401

        return
    except ImportError:
        pass
    import types
    import ctypes
    import contextlib

    mod = types.ModuleType("antenv.axon_hooks")
    _hook_box = [None]

    def set_axon_ntff_profile_hook(h):
        _hook_box[0] = h

    def get_axon_ntff_profile_hook():
        return _hook_box[0]

    mod.set_axon_ntff_profile_hook = set_axon_ntff_profile_hook
    mod.get_axon_ntff_profile_hook = get_axon_ntff_profile_hook

    so_path = "/opt/axon/libaxon_pjrt.so"
    if os.path.exists(so_path):
        try:
            lib = ctypes.CDLL(so_path)
            if hasattr(lib, "axon_start_nrt_profile"):
                lib.axon_start_nrt_profile.argtypes = [
                    ctypes.POINTER(ctypes.c_int64),
                    ctypes.c_size_t,
                ]
                lib.axon_start_nrt_profile.restype = ctypes.c_int64
                lib.axon_stop_nrt_profile.argtypes = [ctypes.c_char_p]
                lib.axon_stop_nrt_profile.restype = ctypes.c_int64

                @contextlib.contextmanager
                def _hook(output_dir, device_ids):
                    import jax as _jax

                    _jax.devices()
                    if device_ids:
                        ids = (ctypes.c_int64 * len(device_ids))(*device_ids)
                        rc = lib.axon_start_nrt_profile(ids, len(device_ids))
                    else:
                        rc = lib.axon_start_nrt_profile(None, 0)
                    if rc != 0:
                        raise RuntimeError(f"axon_start_nrt_profile rc={rc}")
                    try:
                        yield
                    finally:
                        n = lib.axon_stop_nrt_profile(str(output_dir).encode())
                        print(f"profile: {n} file(s) written to {output_dir}", file=sys.stderr)

                _hook_box[0] = _hook
        except OSError:
            pass

    import antenv

    sys.modules["antenv.axon_hooks"] = mod
    antenv.axon_hooks = mod


_ensure_axon_hooks()

# upload_artifacts reaches for a remote bucket; in this sandbox just keep
# artifacts local (only used on the trace path).
_orig_upload = _bass_utils.upload_artifacts


def _safe_upload(tmpdir):
    try:
        return _orig_upload(tmpdir)
    except Exception:
        return str(tmpdir)


_bass_utils.upload_artifacts = _safe_upload

# ---------------------------------------------------------------- constants
E = 8            # experts == cores
H = 2048         # hidden
DFF = 5461       # ffn inner
NDT = 43         # dff tiles of 128 (padded)
DFFP = NDT * 128 # 5504
KTOK = 1024      # expert capacity (top-k per expert)
NTOK = 8192      # total tokens
NKC = H // 128   # 16 K-chunks for gate/up
NTC = 2          # token chunks of 512 (gate/up free dim)
NTT = 8          # token tiles of 128 (down partition dim)
NHC = 4          # H chunks of 512 (down free dim)
EPS = 1e-10

BF16 = mybir.dt.bfloat16
F32 = mybir.dt.float32

# ---------------------------------------------------------------- device IR
def _ffn_body(ctx, tc, xt, wg, wu, wd, bg, bu, bd, gv, y):
    nc = tc.nc

    const_pool = ctx.enter_context(tc.tile_pool(name="const", bufs=1))
    h_pool = ctx.enter_context(tc.tile_pool(name="h", bufs=1))
    out_pool = ctx.enter_context(tc.tile_pool(name="out", bufs=2))
    psum_yp = ctx.enter_context(tc.tile_pool(name="psum_y", bufs=2, space="PSUM"))

    # constants / small tensors
    bg_sb = const_pool.tile([128, NDT], F32)
    nc.gpsimd.dma_start(bg_sb[:], bg[:])
    bu_sb = const_pool.tile([128, NDT], F32)
    nc.gpsimd.dma_start(bu_sb[:], bu[:])
    bd_hi = const_pool.tile([1, H], BF16)
    nc.gpsimd.dma_start(bd_hi[:], bd[0:1, :])
    gv_sb = const_pool.tile([128, NTT], F32)
    nc.gpsimd.dma_start(gv_sb[:], gv[:])
    ones_sb = const_pool.tile([1, 128], BF16)
    nc.vector.memset(ones_sb[:], 1.0)

    # hT[dff, tok] as 43 tiles of [128, 1024] packed along free dim
    h_all = h_pool.tile([128, NDT * KTOK], BF16)

    # wd pool opened early so hc=0 slab loads can overlap late phase B
    wd_pool = ctx.enter_context(tc.tile_pool(name="wdp", bufs=48))
    wd_sbs = {}

    def _load_wd(hc, dc):
        wds = wd_pool.tile([128, 512], BF16, tag="wds")
        nc.sync.dma_start(wds[:], wd[:, hc, dc * 512 : (dc + 1) * 512])
        wd_sbs[(hc, dc)] = wds

    # ---------------- phase B: gate/up + silu*up -> h_all ----------------
    with (
        tc.tile_pool(name="xt", bufs=1) as xt_pool,
        tc.tile_pool(name="guw", bufs=3) as guw_pool,
        tc.tile_pool(name="gus", bufs=2) as gus_pool,
        tc.tile_pool(name="psum_gu", bufs=3, space="PSUM") as psum_gu,
    ):
        xt_sb = xt_pool.tile([128, NKC * KTOK], BF16)
        for kc in range(NKC):
            nc.sync.dma_start(
                xt_sb[:, kc * KTOK : (kc + 1) * KTOK], xt[:, kc * KTOK : (kc + 1) * KTOK]
            )

        for ti in range(NDT):
            wg_sb = guw_pool.tile([128, H], BF16, tag="wgslab")
            nc.gpsimd.dma_start(wg_sb[:], wg[:, ti, :])
            wu_sb = guw_pool.tile([128, H], BF16, tag="wuslab")
            nc.gpsimd.dma_start(wu_sb[:], wu[:, ti, :])
            if ti >= 22 and (ti - 22) * 2 < NDT:
                _load_wd(0, (ti - 22) * 2)
                if (ti - 22) * 2 + 1 < NDT:
                    _load_wd(0, (ti - 22) * 2 + 1)
            for tcx in range(NTC):
                pg = psum_gu.tile([128, 512], F32, tag="pg")
                pu = psum_gu.tile([128, 512], F32, tag="pu")
                for kc in range(NKC):
                    rhs = xt_sb[:, kc * KTOK + tcx * 512 : kc * KTOK + tcx * 512 + 512]
                    nc.tensor.matmul(
                        pg[:],
                        wg_sb[:, kc * 128 : (kc + 1) * 128],
                        rhs,
                        start=(kc == 0),
                        stop=(kc == NKC - 1),
                    )
                for kc in range(NKC):
                    rhs = xt_sb[:, kc * KTOK + tcx * 512 : kc * KTOK + tcx * 512 + 512]
                    nc.tensor.matmul(
                        pu[:],
                        wu_sb[:, kc * 128 : (kc + 1) * 128],
                        rhs,
                        start=(kc == 0),
                        stop=(kc == NKC - 1),
                    )
                gact = gus_pool.tile([128, 512], F32, tag="gact")
                nc.scalar.activation(
                    gact[:],
                    pg[:],
                    mybir.ActivationFunctionType.Silu,
                    bias=bg_sb[:, ti : ti + 1],
                )
                uact = gus_pool.tile([128, 512], F32, tag="uact")
                nc.vector.tensor_scalar_add(uact[:], pu[:], bu_sb[:, ti : ti + 1])
                hsl = h_all[:, ti * KTOK + tcx * 512 : ti * KTOK + tcx * 512 + 512]
                nc.vector.tensor_mul(hsl, gact[:], uact[:])

    # ---------------- phase C: y = (hT^T @ Wd + bd) * G ----------------
    for hc in range(NHC):
        for dc in range(NDT):
            if (hc, dc) not in wd_sbs:
                _load_wd(hc, dc)
        for tt in range(NTT):
            py = psum_yp.tile([128, 512], F32, tag="py")
            for dc in range(NDT):
                lhsT = h_all[:, dc * KTOK + tt * 128 : dc * KTOK + tt * 128 + 128]
                nc.tensor.matmul(
                    py[:], lhsT, wd_sbs[(hc, dc)][:], start=(dc == 0), stop=False
                )
            # += ones^T @ b_down (bf16), closes the group
            nc.tensor.matmul(
                py[:], ones_sb[:], bd_hi[:, hc * 512 : (hc + 1) * 512],
                start=False, stop=True,
            )
            ot = out_pool.tile([128, 512], F32, tag="ot")
            nc.scalar.activation(
                ot[:],
                py[:],
                mybir.ActivationFunctionType.Copy,
                scale=gv_sb[:, tt : tt + 1],
            )
            nc.scalar.dma_start(
                y[tt * 128 : (tt + 1) * 128, hc * 512 : (hc + 1) * 512], ot[:]
            )


_NC_CACHE = {}


def _get_program():
    if "nc" not in _NC_CACHE:
        nc = bacc.Bacc("TRN2", target_bir_lowering=False, debug=False)
        aps = {
            "xt": nc.dram_tensor("xt", [128, NKC * KTOK], BF16, kind="ExternalInput").ap(),
            "wg": nc.dram_tensor("wg", [128, NDT, H], BF16, kind="ExternalInput").ap(),
            "wu": nc.dram_tensor("wu", [128, NDT, H], BF16, kind="ExternalInput").ap(),
            "wd": nc.dram_tensor("wd", [128, NHC, NDT * 512], BF16, kind="ExternalInput").ap(),
            "bg": nc.dram_tensor("bg", [128, NDT], F32, kind="ExternalInput").ap(),
            "bu": nc.dram_tensor("bu", [128, NDT], F32, kind="ExternalInput").ap(),
            "bd": nc.dram_tensor("bd", [2, H], BF16, kind="ExternalInput").ap(),
            "gv": nc.dram_tensor("gv", [128, NTT], F32, kind="ExternalInput").ap(),
            "y": nc.dram_tensor("y", [KTOK, H], F32, kind="ExternalOutput").ap(),
        }
        with tile.TileContext(nc) as tc, ExitStack() as ctx:
            _ffn_body(ctx, tc, **aps)
        nc.compile()
        _NC_CACHE["nc"] = nc
    return _NC_CACHE["nc"]


# ---------------------------------------------------------------- host side
_CPU = jax.devices("cpu")[0]


def _route(xf, router_w, router_b):
    """Exact replica of the reference routing, eagerly on CPU so the top-k
    index selection is bit-identical to the reference."""
    with jax.default_device(_CPU):
        xf = jax.device_put(jnp.asarray(xf), _CPU)
        clean_h = xf @ jax.device_put(jnp.asarray(router_w), _CPU) + jax.device_put(
            jnp.asarray(router_b), _CPU
        )
        u = jax.random.uniform(jax.random.key(42), clean_h.shape, dtype=clean_h.dtype)
        gumbel = -jnp.log(-jnp.log(u + EPS) + EPS)
        h = clean_h + gumbel  # ROUTER_TEMP = 1.0
        Sm = jax.nn.softmax(h, axis=-1)
        Gt, idxt = jax.lax.top_k(Sm.T, KTOK)  # [E, k]
        return np.asarray(Gt), np.asarray(idxt)


def _pack_weights_np(w_gate, w_up, w_down, b_gate, b_up, b_down):
    bf16 = jnp.bfloat16.dtype

    wg_p = np.zeros((H, DFFP), np.float32)
    wg_p[:, :DFF] = w_gate
    wu_p = np.zeros((H, DFFP), np.float32)
    wu_p[:, :DFF] = w_up
    wd_p = np.zeros((DFFP, H), np.float32)
    wd_p[:DFF, :] = w_down

    # [H, DFFP] -> [p, ti, kc*128+m]
    wg_dev = np.ascontiguousarray(
        wg_p.reshape(NKC, 128, NDT, 128).transpose(1, 2, 0, 3).reshape(128, NDT, H)
    ).astype(bf16)
    wu_dev = np.ascontiguousarray(
        wu_p.reshape(NKC, 128, NDT, 128).transpose(1, 2, 0, 3).reshape(128, NDT, H)
    ).astype(bf16)
    # [DFFP, H] -> [p, hc, dc*512+m]
    wd_dev = np.ascontiguousarray(
        wd_p.reshape(NDT, 128, NHC, 512).transpose(1, 2, 0, 3).reshape(128, NHC, NDT * 512)
    ).astype(bf16)

    bgp = np.zeros((DFFP,), np.float32)
    bgp[:DFF] = b_gate
    bup = np.zeros((DFFP,), np.float32)
    bup[:DFF] = b_up
    bg_dev = np.ascontiguousarray(bgp.reshape(NDT, 128).T)
    bu_dev = np.ascontiguousarray(bup.reshape(NDT, 128).T)

    bd_hi = b_down.astype(bf16)
    bd_lo = (b_down - bd_hi.astype(np.float32)).astype(bf16)
    bd_dev = np.ascontiguousarray(np.stack([bd_hi, bd_lo], axis=0))  # [2, H]
    return wg_dev, wu_dev, wd_dev, bg_dev, bu_dev, bd_dev


def _gather_pack_fn(xf, idxt):
    # toks[e] = xf[idxt[e]]  -> xt[e][p, kc*1024+t] = toks[e][t, kc*128+p]
    toks = jnp.take(xf, idxt.reshape(-1), axis=0).reshape(E, KTOK, H)
    xt = toks.reshape(E, KTOK, NKC, 128).transpose(0, 3, 2, 1).reshape(E, 128, NKC * KTOK)
    return xt.astype(jnp.bfloat16)


def _scatter_out_fn(dev_outs, idxt):
    # dev_outs: [E, KTOK, H] (already G-weighted, b_down included)
    yf = jnp.zeros((NTOK, H), jnp.float32)
    yf = yf.at[idxt.reshape(-1)].add(dev_outs.reshape(-1, H))
    return yf


_gather_pack = jax.jit(_gather_pack_fn)
_scatter_out = jax.jit(_scatter_out_fn)

LAST_RESULTS = None


def kernel(x, router_w, router_b, w_gate, b_gate, w_up, b_up, w_down, b_down):
    global LAST_RESULTS
    B, S, _ = x.shape
    xf_np = np.asarray(x, dtype=np.float32).reshape(NTOK, H)

    Gt, idxt = _route(xf_np, np.asarray(router_w), np.asarray(router_b))

    with jax.default_device(_CPU):
        xt_all = np.asarray(
            _gather_pack(jax.device_put(xf_np, _CPU), jax.device_put(idxt, _CPU))
        )

    wg_dev, wu_dev, wd_dev, bg_dev, bu_dev, bd_dev = _pack_weights_np(
        np.asarray(w_gate, np.float32),
        np.asarray(w_up, np.float32),
        np.asarray(w_down, np.float32),
        np.asarray(b_gate, np.float32),
        np.asarray(b_up, np.float32),
        np.asarray(b_down, np.float32),
    )

    # G columns per expert: gv[p, tt] = G[tt*128+p] for expert e; Gt is [E, k]
    gv_all = np.ascontiguousarray(
        Gt.reshape(E, NTT, 128).transpose(0, 2, 1).astype(np.float32)
    )

    nc = _get_program()
    in_maps = []
    for e in range(E):
        in_maps.append(
            {
                "xt": np.ascontiguousarray(xt_all[e]),
                "wg": wg_dev,
                "wu": wu_dev,
                "wd": wd_dev,
                "bg": bg_dev,
                "bu": bu_dev,
                "bd": bd_dev,
                "gv": gv_all[e],
            }
        )
    res = run_bass_kernel_spmd(nc, in_maps, list(range(E)))
    LAST_RESULTS = res

    dev_outs = np.stack([res.results[e]["y"] for e in range(E)], axis=0)

    with jax.default_device(_CPU):
        yf = np.asarray(
            _scatter_out(jax.device_put(dev_outs, _CPU), jax.device_put(idxt, _CPU))
        )
    return yf.reshape(B, S, H)


# revision 24
# speedup vs baseline: 1.0162x; 1.0085x over previous
"""Expert-choice MoE (8 experts, shared SwiGLU weights) on 8 trn2 NeuronCores.

Sharding: expert-parallel, one expert per core. Routing (tiny: 8192x2048x8
matmul + softmax + top-k) runs on host CPU with the exact same jax ops as the
reference so the discrete top-k selection is bit-identical; scatter-add is
order-invariant so only the selected set matters. The heavy FFN
(~69 GFLOP/core) runs on device in bf16 with fp32 PSUM accumulation.

Device kernel per core (expert e):
  phase B: hT[dff, tok] = silu(Wg^T x + bg) * (Wu^T x + bu)   (dff on partitions)
  phase C: y[tok, H]    = (hT^T @ Wd + b_down) * G[:, e]      (tok on partitions)
All operands are packed host-side so every matmul reads natural K-major SBUF
slices; no on-device transposes. b_down enters as a rank-1 (ones x b_down)
matmul accumulated into the same PSUM group, split hi/lo in bf16 for accuracy.
"""

import os
import sys

for _p in ("/opt/trn_rl_repo",):
    if _p not in sys.path and os.path.isdir(_p):
        sys.path.insert(0, _p)

import numpy as np
from contextlib import ExitStack

import jax
import jax.numpy as jnp

import concourse.bass as bass
import concourse.tile as tile
from concourse import bacc, mybir
from concourse import bass_utils as _bass_utils
from concourse.bass_utils import run_bass_kernel_spmd


def _ensure_axon_hooks():
    """Provide antenv.axon_hooks if the image lacks it, so trace=True /
    BASS_TRACE profiling works (and degrades gracefully when it can't)."""
    try:
        import antenv.axon_hooks  # noqa: F401

        return
    except ImportError:
        pass
    import types
    import ctypes
    import contextlib

    mod = types.ModuleType("antenv.axon_hooks")
    _hook_box = [None]

    def set_axon_ntff_profile_hook(h):
        _hook_box[0] = h

    def get_axon_ntff_profile_hook():
        return _hook_box[0]

    mod.set_axon_ntff_profile_hook = set_axon_ntff_profile_hook
    mod.get_axon_ntff_profile_hook = get_axon_ntff_profile_hook

    so_path = "/opt/axon/libaxon_pjrt.so"
    if os.path.exists(so_path):
        try:
            lib = ctypes.CDLL(so_path)
            if hasattr(lib, "axon_start_nrt_profile"):
                lib.axon_start_nrt_profile.argtypes = [
                    ctypes.POINTER(ctypes.c_int64),
                    ctypes.c_size_t,
                ]
                lib.axon_start_nrt_profile.restype = ctypes.c_int64
                lib.axon_stop_nrt_profile.argtypes = [ctypes.c_char_p]
                lib.axon_stop_nrt_profile.restype = ctypes.c_int64

                @contextlib.contextmanager
                def _hook(output_dir, device_ids):
                    import jax as _jax

                    _jax.devices()
                    if device_ids:
                        ids = (ctypes.c_int64 * len(device_ids))(*device_ids)
                        rc = lib.axon_start_nrt_profile(ids, len(device_ids))
                    else:
                        rc = lib.axon_start_nrt_profile(None, 0)
                    if rc != 0:
                        raise RuntimeError(f"axon_start_nrt_profile rc={rc}")
                    try:
                        yield
                    finally:
                        n = lib.axon_stop_nrt_profile(str(output_dir).encode())
                        print(f"profile: {n} file(s) written to {output_dir}", file=sys.stderr)

                _hook_box[0] = _hook
        except OSError:
            pass

    import antenv

    sys.modules["antenv.axon_hooks"] = mod
    antenv.axon_hooks = mod


_ensure_axon_hooks()

# upload_artifacts reaches for a remote bucket; in this sandbox just keep
# artifacts local (only used on the trace path).
_orig_upload = _bass_utils.upload_artifacts


def _safe_upload(tmpdir):
    try:
        return _orig_upload(tmpdir)
    except Exception:
        return str(tmpdir)


_bass_utils.upload_artifacts = _safe_upload

# ---------------------------------------------------------------- constants
E = 8            # experts == cores
H = 2048         # hidden
DFF = 5461       # ffn inner
NDT = 43         # dff tiles of 128 (padded)
DFFP = NDT * 128 # 5504
KTOK = 1024      # expert capacity (top-k per expert)
NTOK = 8192      # total tokens
NKC = H // 128   # 16 K-chunks for gate/up
NTC = 2          # token chunks of 512 (gate/up free dim)
NTT = 8          # token tiles of 128 (down partition dim)
NHC = 4          # H chunks of 512 (down free dim)
EPS = 1e-10

BF16 = mybir.dt.bfloat16
F32 = mybir.dt.float32

# ---------------------------------------------------------------- device IR
def _ffn_body(ctx, tc, xt, wg, wu, wd, bg, bu, bd, gv, y):
    nc = tc.nc

    const_pool = ctx.enter_context(tc.tile_pool(name="const", bufs=1))
    h_pool = ctx.enter_context(tc.tile_pool(name="h", bufs=1))
    out_pool = ctx.enter_context(tc.tile_pool(name="out", bufs=2))
    psum_yp = ctx.enter_context(tc.tile_pool(name="psum_y", bufs=2, space="PSUM"))

    # constants / small tensors
    bg_sb = const_pool.tile([128, NDT], F32)
    nc.gpsimd.dma_start(bg_sb[:], bg[:])
    bu_sb = const_pool.tile([128, NDT], F32)
    nc.gpsimd.dma_start(bu_sb[:], bu[:])
    bd_hi = const_pool.tile([1, H], BF16)
    nc.gpsimd.dma_start(bd_hi[:], bd[0:1, :])
    gv_sb = const_pool.tile([128, NTT], F32)
    nc.gpsimd.dma_start(gv_sb[:], gv[:])
    ones_sb = const_pool.tile([1, 128], BF16)
    nc.vector.memset(ones_sb[:], 1.0)

    # hT[dff, tok] as 43 tiles of [128, 1024] packed along free dim
    h_all = h_pool.tile([128, NDT * KTOK], BF16)

    # wd pool opened early so hc=0 slab loads can overlap late phase B
    wd_pool = ctx.enter_context(tc.tile_pool(name="wdp", bufs=52))
    wd_sbs = {}

    def _load_wd(hc, dc):
        wds = wd_pool.tile([128, 512], BF16, tag="wds")
        eng = nc.sync if dc % 2 == 0 else nc.scalar
        eng.dma_start(wds[:], wd[:, hc, dc * 512 : (dc + 1) * 512])
        wd_sbs[(hc, dc)] = wds

    # ---------------- phase B: gate/up + silu*up -> h_all ----------------
    with (
        tc.tile_pool(name="xt", bufs=1) as xt_pool,
        tc.tile_pool(name="guw", bufs=3) as guw_pool,
        tc.tile_pool(name="gus", bufs=2) as gus_pool,
        tc.tile_pool(name="psum_gu", bufs=3, space="PSUM") as psum_gu,
    ):
        xt_sb = xt_pool.tile([128, NKC * KTOK], BF16)
        for kc in range(NKC):
            nc.sync.dma_start(
                xt_sb[:, kc * KTOK : (kc + 1) * KTOK], xt[:, kc * KTOK : (kc + 1) * KTOK]
            )

        for ti in range(NDT):
            wg_sb = guw_pool.tile([128, H], BF16, tag="wgslab")
            nc.gpsimd.dma_start(wg_sb[:], wg[:, ti, :])
            wu_sb = guw_pool.tile([128, H], BF16, tag="wuslab")
            nc.gpsimd.dma_start(wu_sb[:], wu[:, ti, :])
            if ti >= 22 and (ti - 22) * 2 < NDT:
                _load_wd(0, (ti - 22) * 2)
                if (ti - 22) * 2 + 1 < NDT:
                    _load_wd(0, (ti - 22) * 2 + 1)
            for tcx in range(NTC):
                pg = psum_gu.tile([128, 512], F32, tag="pg")
                pu = psum_gu.tile([128, 512], F32, tag="pu")
                for kc in range(NKC):
                    rhs = xt_sb[:, kc * KTOK + tcx * 512 : kc * KTOK + tcx * 512 + 512]
                    nc.tensor.matmul(
                        pg[:],
                        wg_sb[:, kc * 128 : (kc + 1) * 128],
                        rhs,
                        start=(kc == 0),
                        stop=(kc == NKC - 1),
                    )
                for kc in range(NKC):
                    rhs = xt_sb[:, kc * KTOK + tcx * 512 : kc * KTOK + tcx * 512 + 512]
                    nc.tensor.matmul(
                        pu[:],
                        wu_sb[:, kc * 128 : (kc + 1) * 128],
                        rhs,
                        start=(kc == 0),
                        stop=(kc == NKC - 1),
                    )
                gact = gus_pool.tile([128, 512], BF16, tag="gact")
                nc.scalar.activation(
                    gact[:],
                    pg[:],
                    mybir.ActivationFunctionType.Silu,
                    bias=bg_sb[:, ti : ti + 1],
                )
                uact = gus_pool.tile([128, 512], BF16, tag="uact")
                nc.vector.tensor_scalar_add(uact[:], pu[:], bu_sb[:, ti : ti + 1])
                hsl = h_all[:, ti * KTOK + tcx * 512 : ti * KTOK + tcx * 512 + 512]
                nc.vector.tensor_mul(hsl, gact[:], uact[:])

    # ---------------- phase C: y = (hT^T @ Wd + bd) * G ----------------
    for hc in range(NHC):
        for dc in range(NDT):
            if (hc, dc) not in wd_sbs:
                _load_wd(hc, dc)
        for tt in range(NTT):
            py = psum_yp.tile([128, 512], F32, tag="py")
            for dc in range(NDT):
                lhsT = h_all[:, dc * KTOK + tt * 128 : dc * KTOK + tt * 128 + 128]
                nc.tensor.matmul(
                    py[:], lhsT, wd_sbs[(hc, dc)][:], start=(dc == 0), stop=False
                )
            # += ones^T @ b_down (bf16), closes the group
            nc.tensor.matmul(
                py[:], ones_sb[:], bd_hi[:, hc * 512 : (hc + 1) * 512],
                start=False, stop=True,
            )
            ot = out_pool.tile([128, 512], F32, tag="ot")
            nc.scalar.activation(
                ot[:],
                py[:],
                mybir.ActivationFunctionType.Copy,
                scale=gv_sb[:, tt : tt + 1],
            )
            nc.scalar.dma_start(
                y[tt * 128 : (tt + 1) * 128, hc * 512 : (hc + 1) * 512], ot[:]
            )


_NC_CACHE = {}


def _get_program():
    if "nc" not in _NC_CACHE:
        nc = bacc.Bacc("TRN2", target_bir_lowering=False, debug=False)
        aps = {
            "xt": nc.dram_tensor("xt", [128, NKC * KTOK], BF16, kind="ExternalInput").ap(),
            "wg": nc.dram_tensor("wg", [128, NDT, H], BF16, kind="ExternalInput").ap(),
            "wu": nc.dram_tensor("wu", [128, NDT, H], BF16, kind="ExternalInput").ap(),
            "wd": nc.dram_tensor("wd", [128, NHC, NDT * 512], BF16, kind="ExternalInput").ap(),
            "bg": nc.dram_tensor("bg", [128, NDT], F32, kind="ExternalInput").ap(),
            "bu": nc.dram_tensor("bu", [128, NDT], F32, kind="ExternalInput").ap(),
            "bd": nc.dram_tensor("bd", [2, H], BF16, kind="ExternalInput").ap(),
            "gv": nc.dram_tensor("gv", [128, NTT], F32, kind="ExternalInput").ap(),
            "y": nc.dram_tensor("y", [KTOK, H], F32, kind="ExternalOutput").ap(),
        }
        with tile.TileContext(nc) as tc, ExitStack() as ctx:
            _ffn_body(ctx, tc, **aps)
        nc.compile()
        _NC_CACHE["nc"] = nc
    return _NC_CACHE["nc"]


# ---------------------------------------------------------------- host side
_CPU = jax.devices("cpu")[0]


def _route(xf, router_w, router_b):
    """Exact replica of the reference routing, eagerly on CPU so the top-k
    index selection is bit-identical to the reference."""
    with jax.default_device(_CPU):
        xf = jax.device_put(jnp.asarray(xf), _CPU)
        clean_h = xf @ jax.device_put(jnp.asarray(router_w), _CPU) + jax.device_put(
            jnp.asarray(router_b), _CPU
        )
        u = jax.random.uniform(jax.random.key(42), clean_h.shape, dtype=clean_h.dtype)
        gumbel = -jnp.log(-jnp.log(u + EPS) + EPS)
        h = clean_h + gumbel  # ROUTER_TEMP = 1.0
        Sm = jax.nn.softmax(h, axis=-1)
        Gt, idxt = jax.lax.top_k(Sm.T, KTOK)  # [E, k]
        return np.asarray(Gt), np.asarray(idxt)


def _pack_weights_np(w_gate, w_up, w_down, b_gate, b_up, b_down):
    bf16 = jnp.bfloat16.dtype

    wg_p = np.zeros((H, DFFP), np.float32)
    wg_p[:, :DFF] = w_gate
    wu_p = np.zeros((H, DFFP), np.float32)
    wu_p[:, :DFF] = w_up
    wd_p = np.zeros((DFFP, H), np.float32)
    wd_p[:DFF, :] = w_down

    # [H, DFFP] -> [p, ti, kc*128+m]
    wg_dev = np.ascontiguousarray(
        wg_p.reshape(NKC, 128, NDT, 128).transpose(1, 2, 0, 3).reshape(128, NDT, H)
    ).astype(bf16)
    wu_dev = np.ascontiguousarray(
        wu_p.reshape(NKC, 128, NDT, 128).transpose(1, 2, 0, 3).reshape(128, NDT, H)
    ).astype(bf16)
    # [DFFP, H] -> [p, hc, dc*512+m]
    wd_dev = np.ascontiguousarray(
        wd_p.reshape(NDT, 128, NHC, 512).transpose(1, 2, 0, 3).reshape(128, NHC, NDT * 512)
    ).astype(bf16)

    bgp = np.zeros((DFFP,), np.float32)
    bgp[:DFF] = b_gate
    bup = np.zeros((DFFP,), np.float32)
    bup[:DFF] = b_up
    bg_dev = np.ascontiguousarray(bgp.reshape(NDT, 128).T)
    bu_dev = np.ascontiguousarray(bup.reshape(NDT, 128).T)

    bd_hi = b_down.astype(bf16)
    bd_lo = (b_down - bd_hi.astype(np.float32)).astype(bf16)
    bd_dev = np.ascontiguousarray(np.stack([bd_hi, bd_lo], axis=0))  # [2, H]
    return wg_dev, wu_dev, wd_dev, bg_dev, bu_dev, bd_dev


def _gather_pack_fn(xf, idxt):
    # toks[e] = xf[idxt[e]]  -> xt[e][p, kc*1024+t] = toks[e][t, kc*128+p]
    toks = jnp.take(xf, idxt.reshape(-1), axis=0).reshape(E, KTOK, H)
    xt = toks.reshape(E, KTOK, NKC, 128).transpose(0, 3, 2, 1).reshape(E, 128, NKC * KTOK)
    return xt.astype(jnp.bfloat16)


def _scatter_out_fn(dev_outs, idxt):
    # dev_outs: [E, KTOK, H] (already G-weighted, b_down included)
    yf = jnp.zeros((NTOK, H), jnp.float32)
    yf = yf.at[idxt.reshape(-1)].add(dev_outs.reshape(-1, H))
    return yf


_gather_pack = jax.jit(_gather_pack_fn)
_scatter_out = jax.jit(_scatter_out_fn)

LAST_RESULTS = None


def kernel(x, router_w, router_b, w_gate, b_gate, w_up, b_up, w_down, b_down):
    global LAST_RESULTS
    B, S, _ = x.shape
    xf_np = np.asarray(x, dtype=np.float32).reshape(NTOK, H)

    Gt, idxt = _route(xf_np, np.asarray(router_w), np.asarray(router_b))

    with jax.default_device(_CPU):
        xt_all = np.asarray(
            _gather_pack(jax.device_put(xf_np, _CPU), jax.device_put(idxt, _CPU))
        )

    wg_dev, wu_dev, wd_dev, bg_dev, bu_dev, bd_dev = _pack_weights_np(
        np.asarray(w_gate, np.float32),
        np.asarray(w_up, np.float32),
        np.asarray(w_down, np.float32),
        np.asarray(b_gate, np.float32),
        np.asarray(b_up, np.float32),
        np.asarray(b_down, np.float32),
    )

    # G columns per expert: gv[p, tt] = G[tt*128+p] for expert e; Gt is [E, k]
    gv_all = np.ascontiguousarray(
        Gt.reshape(E, NTT, 128).transpose(0, 2, 1).astype(np.float32)
    )

    nc = _get_program()
    in_maps = []
    for e in range(E):
        in_maps.append(
            {
                "xt": np.ascontiguousarray(xt_all[e]),
                "wg": wg_dev,
                "wu": wu_dev,
                "wd": wd_dev,
                "bg": bg_dev,
                "bu": bu_dev,
                "bd": bd_dev,
                "gv": gv_all[e],
            }
        )
    res = run_bass_kernel_spmd(nc, in_maps, list(range(E)))
    LAST_RESULTS = res

    dev_outs = np.stack([res.results[e]["y"] for e in range(E)], axis=0)

    with jax.default_device(_CPU):
        yf = np.asarray(
            _scatter_out(jax.device_put(dev_outs, _CPU), jax.device_put(idxt, _CPU))
        )
    return yf.reshape(B, S, H)


# revision 26
# speedup vs baseline: 1.0264x; 1.0100x over previous
"""Expert-choice MoE (8 experts, shared SwiGLU weights) on 8 trn2 NeuronCores.

Sharding: expert-parallel, one expert per core. Routing (tiny: 8192x2048x8
matmul + softmax + top-k) runs on host CPU with the exact same jax ops as the
reference so the discrete top-k selection is bit-identical; scatter-add is
order-invariant so only the selected set matters. The heavy FFN
(~69 GFLOP/core) runs on device in bf16 with fp32 PSUM accumulation.

Device kernel per core (expert e):
  phase B: hT[dff, tok] = silu(Wg^T x + bg) * (Wu^T x + bu)   (dff on partitions)
  phase C: y[tok, H]    = (hT^T @ Wd + b_down) * G[:, e]      (tok on partitions)
All operands are packed host-side so every matmul reads natural K-major SBUF
slices; no on-device transposes. b_down enters as a rank-1 (ones x b_down)
matmul accumulated into the same PSUM group, split hi/lo in bf16 for accuracy.
"""

import os
import sys

for _p in ("/opt/trn_rl_repo",):
    if _p not in sys.path and os.path.isdir(_p):
        sys.path.insert(0, _p)

import numpy as np
from contextlib import ExitStack

import jax
import jax.numpy as jnp

import concourse.bass as bass
import concourse.tile as tile
from concourse import bacc, mybir
from concourse import bass_utils as _bass_utils
from concourse.bass_utils import run_bass_kernel_spmd


def _ensure_axon_hooks():
    """Provide antenv.axon_hooks if the image lacks it, so trace=True /
    BASS_TRACE profiling works (and degrades gracefully when it can't)."""
    try:
        import antenv.axon_hooks  # noqa: F401

        return
    except ImportError:
        pass
    import types
    import ctypes
    import contextlib

    mod = types.ModuleType("antenv.axon_hooks")
    _hook_box = [None]

    def set_axon_ntff_profile_hook(h):
        _hook_box[0] = h

    def get_axon_ntff_profile_hook():
        return _hook_box[0]

    mod.set_axon_ntff_profile_hook = set_axon_ntff_profile_hook
    mod.get_axon_ntff_profile_hook = get_axon_ntff_profile_hook

    so_path = "/opt/axon/libaxon_pjrt.so"
    if os.path.exists(so_path):
        try:
            lib = ctypes.CDLL(so_path)
            if hasattr(lib, "axon_start_nrt_profile"):
                lib.axon_start_nrt_profile.argtypes = [
                    ctypes.POINTER(ctypes.c_int64),
                    ctypes.c_size_t,
                ]
                lib.axon_start_nrt_profile.restype = ctypes.c_int64
                lib.axon_stop_nrt_profile.argtypes = [ctypes.c_char_p]
                lib.axon_stop_nrt_profile.restype = ctypes.c_int64

                @contextlib.contextmanager
                def _hook(output_dir, device_ids):
                    import jax as _jax

                    _jax.devices()
                    if device_ids:
                        ids = (ctypes.c_int64 * len(device_ids))(*device_ids)
                        rc = lib.axon_start_nrt_profile(ids, len(device_ids))
                    else:
                        rc = lib.axon_start_nrt_profile(None, 0)
                    if rc != 0:
                        raise RuntimeError(f"axon_start_nrt_profile rc={rc}")
                    try:
                        yield
                    finally:
                        n = lib.axon_stop_nrt_profile(str(output_dir).encode())
                        print(f"profile: {n} file(s) written to {output_dir}", file=sys.stderr)

                _hook_box[0] = _hook
        except OSError:
            pass

    import antenv

    sys.modules["antenv.axon_hooks"] = mod
    antenv.axon_hooks = mod


_ensure_axon_hooks()

# upload_artifacts reaches for a remote bucket; in this sandbox just keep
# artifacts local (only used on the trace path).
_orig_upload = _bass_utils.upload_artifacts


def _safe_upload(tmpdir):
    try:
        return _orig_upload(tmpdir)
    except Exception:
        return str(tmpdir)


_bass_utils.upload_artifacts = _safe_upload

# ---------------------------------------------------------------- constants
E = 8            # experts == cores
H = 2048         # hidden
DFF = 5461       # ffn inner
NDT = 43         # dff tiles of 128 (padded)
DFFP = NDT * 128 # 5504
KTOK = 1024      # expert capacity (top-k per expert)
NTOK = 8192      # total tokens
NKC = H // 128   # 16 K-chunks for gate/up
NTC = 2          # token chunks of 512 (gate/up free dim)
NTT = 8          # token tiles of 128 (down partition dim)
NHC = 4          # H chunks of 512 (down free dim)
EPS = 1e-10

BF16 = mybir.dt.bfloat16
F32 = mybir.dt.float32

# ---------------------------------------------------------------- device IR
def _ffn_body(ctx, tc, xt, wg, wu, wd, bg, bu, gv, y):
    nc = tc.nc

    const_pool = ctx.enter_context(tc.tile_pool(name="const", bufs=1))
    h_pool = ctx.enter_context(tc.tile_pool(name="h", bufs=1))
    out_pool = ctx.enter_context(tc.tile_pool(name="out", bufs=2))
    psum_yp = ctx.enter_context(tc.tile_pool(name="psum_y", bufs=2, space="PSUM"))

    # constants / small tensors
    bg_sb = const_pool.tile([128, NDT], F32)
    nc.gpsimd.dma_start(bg_sb[:], bg[:])
    bu_sb = const_pool.tile([128, NDT], F32)
    nc.gpsimd.dma_start(bu_sb[:], bu[:])
    gv_sb = const_pool.tile([128, NTT], F32)
    nc.gpsimd.dma_start(gv_sb[:], gv[:])

    # hT[dff, tok] as 43 tiles of [128, 1024] packed along free dim
    h_all = h_pool.tile([128, NDT * KTOK], BF16)

    # wd pool opened early so hc=0 slab loads can overlap late phase B
    wd_pool = ctx.enter_context(tc.tile_pool(name="wdp", bufs=52))
    wd_sbs = {}

    def _load_wd(hc, dc):
        wds = wd_pool.tile([128, 512], BF16, tag="wds")
        eng = nc.sync if dc % 2 == 0 else nc.scalar
        eng.dma_start(wds[:], wd[:, hc, dc * 512 : (dc + 1) * 512])
        wd_sbs[(hc, dc)] = wds

    # ---------------- phase B: gate/up + silu*up -> h_all ----------------
    with (
        tc.tile_pool(name="xt", bufs=1) as xt_pool,
        tc.tile_pool(name="guw", bufs=3) as guw_pool,
        tc.tile_pool(name="gus", bufs=2) as gus_pool,
        tc.tile_pool(name="psum_gu", bufs=3, space="PSUM") as psum_gu,
    ):
        xt_sb = xt_pool.tile([128, NKC * KTOK], BF16)
        for kc in range(NKC):
            nc.sync.dma_start(
                xt_sb[:, kc * KTOK : (kc + 1) * KTOK], xt[:, kc * KTOK : (kc + 1) * KTOK]
            )

        for ti in range(NDT):
            wg_sb = guw_pool.tile([128, H], BF16, tag="wgslab")
            nc.gpsimd.dma_start(wg_sb[:], wg[:, ti, :])
            wu_sb = guw_pool.tile([128, H], BF16, tag="wuslab")
            nc.gpsimd.dma_start(wu_sb[:], wu[:, ti, :])
            if ti >= 22 and (ti - 22) * 2 < NDT:
                _load_wd(0, (ti - 22) * 2)
                if (ti - 22) * 2 + 1 < NDT:
                    _load_wd(0, (ti - 22) * 2 + 1)
            for tcx in range(NTC):
                pg = psum_gu.tile([128, 512], F32, tag="pg")
                pu = psum_gu.tile([128, 512], F32, tag="pu")
                kcs = list(range(NKC))
                if ti == 0 and tcx == 0:
                    kcs = kcs[::-1]  # first MM deps on last xt chunk: start gapless
                for j, kc in enumerate(kcs):
                    rhs = xt_sb[:, kc * KTOK + tcx * 512 : kc * KTOK + tcx * 512 + 512]
                    nc.tensor.matmul(
                        pg[:],
                        wg_sb[:, kc * 128 : (kc + 1) * 128],
                        rhs,
                        start=(j == 0),
                        stop=(j == NKC - 1),
                    )
                for kc in range(NKC):
                    rhs = xt_sb[:, kc * KTOK + tcx * 512 : kc * KTOK + tcx * 512 + 512]
                    nc.tensor.matmul(
                        pu[:],
                        wu_sb[:, kc * 128 : (kc + 1) * 128],
                        rhs,
                        start=(kc == 0),
                        stop=(kc == NKC - 1),
                    )
                gact = gus_pool.tile([128, 512], BF16, tag="gact")
                nc.scalar.activation(
                    gact[:],
                    pg[:],
                    mybir.ActivationFunctionType.Silu,
                    bias=bg_sb[:, ti : ti + 1],
                )
                uact = gus_pool.tile([128, 512], BF16, tag="uact")
                nc.vector.tensor_scalar_add(uact[:], pu[:], bu_sb[:, ti : ti + 1])
                hsl = h_all[:, ti * KTOK + tcx * 512 : ti * KTOK + tcx * 512 + 512]
                nc.vector.tensor_mul(hsl, gact[:], uact[:])

    # ---------------- phase C: y = (hT^T @ Wd + b_down via ones-row) * G ----
    for hc in range(NHC):
        for dc in range(NDT):
            if (hc, dc) not in wd_sbs:
                _load_wd(hc, dc)
        for tt in range(NTT):
            py = psum_yp.tile([128, 512], F32, tag="py")
            for dc in range(NDT):
                lhsT = h_all[:, dc * KTOK + tt * 128 : dc * KTOK + tt * 128 + 128]
                nc.tensor.matmul(
                    py[:], lhsT, wd_sbs[(hc, dc)][:], start=(dc == 0),
                    stop=(dc == NDT - 1),
                )
            ot = out_pool.tile([128, 512], F32, tag="ot")
            nc.scalar.activation(
                ot[:],
                py[:],
                mybir.ActivationFunctionType.Copy,
                scale=gv_sb[:, tt : tt + 1],
            )
            nc.scalar.dma_start(
                y[tt * 128 : (tt + 1) * 128, hc * 512 : (hc + 1) * 512], ot[:]
            )


_NC_CACHE = {}


def _get_program():
    if "nc" not in _NC_CACHE:
        nc = bacc.Bacc("TRN2", target_bir_lowering=False, debug=False)
        aps = {
            "xt": nc.dram_tensor("xt", [128, NKC * KTOK], BF16, kind="ExternalInput").ap(),
            "wg": nc.dram_tensor("wg", [128, NDT, H], BF16, kind="ExternalInput").ap(),
            "wu": nc.dram_tensor("wu", [128, NDT, H], BF16, kind="ExternalInput").ap(),
            "wd": nc.dram_tensor("wd", [128, NHC, NDT * 512], BF16, kind="ExternalInput").ap(),
            "bg": nc.dram_tensor("bg", [128, NDT], F32, kind="ExternalInput").ap(),
            "bu": nc.dram_tensor("bu", [128, NDT], F32, kind="ExternalInput").ap(),
            "gv": nc.dram_tensor("gv", [128, NTT], F32, kind="ExternalInput").ap(),
            "y": nc.dram_tensor("y", [KTOK, H], F32, kind="ExternalOutput").ap(),
        }
        with tile.TileContext(nc) as tc, ExitStack() as ctx:
            _ffn_body(ctx, tc, **aps)
        nc.compile()
        _NC_CACHE["nc"] = nc
    return _NC_CACHE["nc"]


# ---------------------------------------------------------------- host side
_CPU = jax.devices("cpu")[0]


def _route(xf, router_w, router_b):
    """Exact replica of the reference routing, eagerly on CPU so the top-k
    index selection is bit-identical to the reference."""
    with jax.default_device(_CPU):
        xf = jax.device_put(jnp.asarray(xf), _CPU)
        clean_h = xf @ jax.device_put(jnp.asarray(router_w), _CPU) + jax.device_put(
            jnp.asarray(router_b), _CPU
        )
        u = jax.random.uniform(jax.random.key(42), clean_h.shape, dtype=clean_h.dtype)
        gumbel = -jnp.log(-jnp.log(u + EPS) + EPS)
        h = clean_h + gumbel  # ROUTER_TEMP = 1.0
        Sm = jax.nn.softmax(h, axis=-1)
        Gt, idxt = jax.lax.top_k(Sm.T, KTOK)  # [E, k]
        return np.asarray(Gt), np.asarray(idxt)


def _pack_weights_np(w_gate, w_up, w_down, b_gate, b_up, b_down):
    bf16 = jnp.bfloat16.dtype

    wg_p = np.zeros((H, DFFP), np.float32)
    wg_p[:, :DFF] = w_gate
    wu_p = np.zeros((H, DFFP), np.float32)
    wu_p[:, :DFF] = w_up
    wd_p = np.zeros((DFFP, H), np.float32)
    wd_p[:DFF, :] = w_down
    wd_p[DFF, :] = b_down  # ones-row in h makes the matmul add b_down

    # [H, DFFP] -> [p, ti, kc*128+m]
    wg_dev = np.ascontiguousarray(
        wg_p.reshape(NKC, 128, NDT, 128).transpose(1, 2, 0, 3).reshape(128, NDT, H)
    ).astype(bf16)
    wu_dev = np.ascontiguousarray(
        wu_p.reshape(NKC, 128, NDT, 128).transpose(1, 2, 0, 3).reshape(128, NDT, H)
    ).astype(bf16)
    # [DFFP, H] -> [p, hc, dc*512+m]
    wd_dev = np.ascontiguousarray(
        wd_p.reshape(NDT, 128, NHC, 512).transpose(1, 2, 0, 3).reshape(128, NHC, NDT * 512)
    ).astype(bf16)

    bgp = np.zeros((DFFP,), np.float32)
    bgp[:DFF] = b_gate
    bup = np.zeros((DFFP,), np.float32)
    bup[:DFF] = b_up
    # pad slot DFF: silu(16)*0.0625 == 1.0 exactly in bf16 -> ones-row in h
    # whose contraction against wd's b_down row adds the bias for free.
    bgp[DFF] = 16.0
    bup[DFF] = 0.0625
    bg_dev = np.ascontiguousarray(bgp.reshape(NDT, 128).T)
    bu_dev = np.ascontiguousarray(bup.reshape(NDT, 128).T)

    return wg_dev, wu_dev, wd_dev, bg_dev, bu_dev


def _gather_pack_fn(xf, idxt):
    # toks[e] = xf[idxt[e]]  -> xt[e][p, kc*1024+t] = toks[e][t, kc*128+p]
    toks = jnp.take(xf, idxt.reshape(-1), axis=0).reshape(E, KTOK, H)
    xt = toks.reshape(E, KTOK, NKC, 128).transpose(0, 3, 2, 1).reshape(E, 128, NKC * KTOK)
    return xt.astype(jnp.bfloat16)


def _scatter_out_fn(dev_outs, idxt):
    # dev_outs: [E, KTOK, H] (already G-weighted, b_down included)
    yf = jnp.zeros((NTOK, H), jnp.float32)
    yf = yf.at[idxt.reshape(-1)].add(dev_outs.reshape(-1, H))
    return yf


_gather_pack = jax.jit(_gather_pack_fn)
_scatter_out = jax.jit(_scatter_out_fn)

LAST_RESULTS = None


def kernel(x, router_w, router_b, w_gate, b_gate, w_up, b_up, w_down, b_down):
    global LAST_RESULTS
    B, S, _ = x.shape
    xf_np = np.asarray(x, dtype=np.float32).reshape(NTOK, H)

    Gt, idxt = _route(xf_np, np.asarray(router_w), np.asarray(router_b))

    with jax.default_device(_CPU):
        xt_all = np.asarray(
            _gather_pack(jax.device_put(xf_np, _CPU), jax.device_put(idxt, _CPU))
        )

    wg_dev, wu_dev, wd_dev, bg_dev, bu_dev = _pack_weights_np(
        np.asarray(w_gate, np.float32),
        np.asarray(w_up, np.float32),
        np.asarray(w_down, np.float32),
        np.asarray(b_gate, np.float32),
        np.asarray(b_up, np.float32),
        np.asarray(b_down, np.float32),
    )

    # G columns per expert: gv[p, tt] = G[tt*128+p] for expert e; Gt is [E, k]
    gv_all = np.ascontiguousarray(
        Gt.reshape(E, NTT, 128).transpose(0, 2, 1).astype(np.float32)
    )

    nc = _get_program()
    in_maps = []
    for e in range(E):
        in_maps.append(
            {
                "xt": np.ascontiguousarray(xt_all[e]),
                "wg": wg_dev,
                "wu": wu_dev,
                "wd": wd_dev,
                "bg": bg_dev,
                "bu": bu_dev,
                "gv": gv_all[e],
            }
        )
    res = run_bass_kernel_spmd(nc, in_maps, list(range(E)))
    LAST_RESULTS = res

    dev_outs = np.stack([res.results[e]["y"] for e in range(E)], axis=0)

    with jax.default_device(_CPU):
        yf = np.asarray(
            _scatter_out(jax.device_put(dev_outs, _CPU), jax.device_put(idxt, _CPU))
        )
    return yf.reshape(B, S, H)


# revision 28
# speedup vs baseline: 1.0512x; 1.0241x over previous
"""Expert-choice MoE (8 experts, shared SwiGLU weights) on 8 trn2 NeuronCores.

Sharding: expert-parallel, one expert per core. Routing (tiny: 8192x2048x8
matmul + softmax + top-k) runs on host CPU with the exact same jax ops as the
reference so the discrete top-k selection is bit-identical; scatter-add is
order-invariant so only the selected set matters. The heavy FFN
(~69 GFLOP/core) runs on device in bf16 with fp32 PSUM accumulation.

Device kernel per core (expert e):
  phase B: hT[dff, tok] = silu(Wg^T x + bg) * (Wu^T x + bu)   (dff on partitions)
  phase C: y[tok, H]    = (hT^T @ Wd + b_down) * G[:, e]      (tok on partitions)
All operands are packed host-side so every matmul reads natural K-major SBUF
slices; no on-device transposes. b_down enters as a rank-1 (ones x b_down)
matmul accumulated into the same PSUM group, split hi/lo in bf16 for accuracy.
"""

import os
import sys

for _p in ("/opt/trn_rl_repo",):
    if _p not in sys.path and os.path.isdir(_p):
        sys.path.insert(0, _p)

import numpy as np
from contextlib import ExitStack

import jax
import jax.numpy as jnp

import concourse.bass as bass
import concourse.tile as tile
from concourse import bacc, mybir
from concourse import bass_utils as _bass_utils
from concourse.bass_utils import run_bass_kernel_spmd


def _ensure_axon_hooks():
    """Provide antenv.axon_hooks if the image lacks it, so trace=True /
    BASS_TRACE profiling works (and degrades gracefully when it can't)."""
    try:
        import antenv.axon_hooks  # noqa: F401

        return
    except ImportError:
        pass
    import types
    import ctypes
    import contextlib

    mod = types.ModuleType("antenv.axon_hooks")
    _hook_box = [None]

    def set_axon_ntff_profile_hook(h):
        _hook_box[0] = h

    def get_axon_ntff_profile_hook():
        return _hook_box[0]

    mod.set_axon_ntff_profile_hook = set_axon_ntff_profile_hook
    mod.get_axon_ntff_profile_hook = get_axon_ntff_profile_hook

    so_path = "/opt/axon/libaxon_pjrt.so"
    if os.path.exists(so_path):
        try:
            lib = ctypes.CDLL(so_path)
            if hasattr(lib, "axon_start_nrt_profile"):
                lib.axon_start_nrt_profile.argtypes = [
                    ctypes.POINTER(ctypes.c_int64),
                    ctypes.c_size_t,
                ]
                lib.axon_start_nrt_profile.restype = ctypes.c_int64
                lib.axon_stop_nrt_profile.argtypes = [ctypes.c_char_p]
                lib.axon_stop_nrt_profile.restype = ctypes.c_int64

                @contextlib.contextmanager
                def _hook(output_dir, device_ids):
                    import jax as _jax

                    _jax.devices()
                    if device_ids:
                        ids = (ctypes.c_int64 * len(device_ids))(*device_ids)
                        rc = lib.axon_start_nrt_profile(ids, len(device_ids))
                    else:
                        rc = lib.axon_start_nrt_profile(None, 0)
                    if rc != 0:
                        raise RuntimeError(f"axon_start_nrt_profile rc={rc}")
                    try:
                        yield
                    finally:
                        n = lib.axon_stop_nrt_profile(str(output_dir).encode())
                        print(f"profile: {n} file(s) written to {output_dir}", file=sys.stderr)

                _hook_box[0] = _hook
        except OSError:
            pass

    import antenv

    sys.modules["antenv.axon_hooks"] = mod
    antenv.axon_hooks = mod


_ensure_axon_hooks()

# upload_artifacts reaches for a remote bucket; in this sandbox just keep
# artifacts local (only used on the trace path).
_orig_upload = _bass_utils.upload_artifacts


def _safe_upload(tmpdir):
    try:
        return _orig_upload(tmpdir)
    except Exception:
        return str(tmpdir)


_bass_utils.upload_artifacts = _safe_upload

# ---------------------------------------------------------------- constants
E = 8            # experts == cores
H = 2048         # hidden
DFF = 5461       # ffn inner
NDT = 43         # dff tiles of 128 (padded)
DFFP = NDT * 128 # 5504
KTOK = 1024      # expert capacity (top-k per expert)
NTOK = 8192      # total tokens
NKC = H // 128   # 16 K-chunks for gate/up
NTC = 2          # token chunks of 512 (gate/up free dim)
NTT = 8          # token tiles of 128 (down partition dim)
NHC = 4          # H chunks of 512 (down free dim)
EPS = 1e-10

BF16 = mybir.dt.bfloat16
F32 = mybir.dt.float32

# ---------------------------------------------------------------- device IR
def _ffn_body(ctx, tc, xt, wg, wu, wd, bg, bu, gv, y):
    nc = tc.nc

    const_pool = ctx.enter_context(tc.tile_pool(name="const", bufs=1))
    h_pool = ctx.enter_context(tc.tile_pool(name="h", bufs=1))
    out_pool = ctx.enter_context(tc.tile_pool(name="out", bufs=2))
    psum_yp = ctx.enter_context(tc.tile_pool(name="psum_y", bufs=2, space="PSUM"))

    # constants / small tensors
    bg_sb = const_pool.tile([128, NDT], F32)
    nc.sync.dma_start(bg_sb[:], bg[:])
    bu_sb = const_pool.tile([128, NDT], F32)
    nc.scalar.dma_start(bu_sb[:], bu[:])
    gv_sb = const_pool.tile([128, NTT], F32)
    nc.sync.dma_start(gv_sb[:], gv[:])

    # hT[dff, tok] as 43 tiles of [128, 1024] packed along free dim
    h_all = h_pool.tile([128, NDT * KTOK], BF16)

    # wd pool opened early so hc=0 slab loads can overlap late phase B
    wd_pool = ctx.enter_context(tc.tile_pool(name="wdp", bufs=56))
    wd_sbs = {}

    def _load_wd(hc, dc):
        wds = wd_pool.tile([128, 512], BF16, tag="wds")
        eng = nc.sync if dc % 2 == 0 else nc.scalar
        eng.dma_start(wds[:], wd[:, hc, dc * 512 : (dc + 1) * 512])
        wd_sbs[(hc, dc)] = wds

    # ---------------- phase B: gate/up + silu*up -> h_all ----------------
    with (
        tc.tile_pool(name="xt", bufs=1) as xt_pool,
        tc.tile_pool(name="guw", bufs=3) as guw_pool,
        tc.tile_pool(name="gus", bufs=2) as gus_pool,
        tc.tile_pool(name="psum_gu", bufs=3, space="PSUM") as psum_gu,
    ):
        xt_sb = xt_pool.tile([128, NKC * KTOK], BF16)
        for kc in range(NKC):
            nc.sync.dma_start(
                xt_sb[:, kc * KTOK : (kc + 1) * KTOK], xt[:, kc * KTOK : (kc + 1) * KTOK]
            )

        for ti in range(NDT):
            wg_sb = guw_pool.tile([128, H], BF16, tag="wgslab")
            nc.sync.dma_start(wg_sb[:], wg[:, ti, :])
            wu_sb = guw_pool.tile([128, H], BF16, tag="wuslab")
            nc.scalar.dma_start(wu_sb[:], wu[:, ti, :])
            if ti >= 22 and (ti - 22) * 2 < NDT:
                _load_wd(0, (ti - 22) * 2)
                if (ti - 22) * 2 + 1 < NDT:
                    _load_wd(0, (ti - 22) * 2 + 1)
            for tcx in range(NTC):
                pg = psum_gu.tile([128, 512], F32, tag="pg")
                pu = psum_gu.tile([128, 512], F32, tag="pu")
                kcs = list(range(NKC))
                if ti == 0 and tcx == 0:
                    kcs = kcs[::-1]  # first MM deps on last xt chunk: start gapless
                for j, kc in enumerate(kcs):
                    rhs = xt_sb[:, kc * KTOK + tcx * 512 : kc * KTOK + tcx * 512 + 512]
                    nc.tensor.matmul(
                        pg[:],
                        wg_sb[:, kc * 128 : (kc + 1) * 128],
                        rhs,
                        start=(j == 0),
                        stop=(j == NKC - 1),
                    )
                for kc in range(NKC):
                    rhs = xt_sb[:, kc * KTOK + tcx * 512 : kc * KTOK + tcx * 512 + 512]
                    nc.tensor.matmul(
                        pu[:],
                        wu_sb[:, kc * 128 : (kc + 1) * 128],
                        rhs,
                        start=(kc == 0),
                        stop=(kc == NKC - 1),
                    )
                gact = gus_pool.tile([128, 512], BF16, tag="gact")
                nc.scalar.activation(
                    gact[:],
                    pg[:],
                    mybir.ActivationFunctionType.Silu,
                    bias=bg_sb[:, ti : ti + 1],
                )
                uact = gus_pool.tile([128, 512], BF16, tag="uact")
                nc.vector.tensor_scalar_add(uact[:], pu[:], bu_sb[:, ti : ti + 1])
                hsl = h_all[:, ti * KTOK + tcx * 512 : ti * KTOK + tcx * 512 + 512]
                nc.vector.tensor_mul(hsl, gact[:], uact[:])

    # ---------------- phase C: y = (hT^T @ Wd + b_down via ones-row) * G ----
    for hc in range(NHC):
        for dc in range(NDT):
            if (hc, dc) not in wd_sbs:
                _load_wd(hc, dc)
        for tt in range(NTT):
            py = psum_yp.tile([128, 512], F32, tag="py")
            for dc in range(NDT):
                lhsT = h_all[:, dc * KTOK + tt * 128 : dc * KTOK + tt * 128 + 128]
                nc.tensor.matmul(
                    py[:], lhsT, wd_sbs[(hc, dc)][:], start=(dc == 0),
                    stop=(dc == NDT - 1),
                )
            ot = out_pool.tile([128, 512], F32, tag="ot")
            nc.scalar.activation(
                ot[:],
                py[:],
                mybir.ActivationFunctionType.Copy,
                scale=gv_sb[:, tt : tt + 1],
            )
            nc.scalar.dma_start(
                y[tt * 128 : (tt + 1) * 128, hc * 512 : (hc + 1) * 512], ot[:]
            )


_NC_CACHE = {}


def _get_program():
    if "nc" not in _NC_CACHE:
        nc = bacc.Bacc("TRN2", target_bir_lowering=False, debug=False)
        aps = {
            "xt": nc.dram_tensor("xt", [128, NKC * KTOK], BF16, kind="ExternalInput").ap(),
            "wg": nc.dram_tensor("wg", [128, NDT, H], BF16, kind="ExternalInput").ap(),
            "wu": nc.dram_tensor("wu", [128, NDT, H], BF16, kind="ExternalInput").ap(),
            "wd": nc.dram_tensor("wd", [128, NHC, NDT * 512], BF16, kind="ExternalInput").ap(),
            "bg": nc.dram_tensor("bg", [128, NDT], F32, kind="ExternalInput").ap(),
            "bu": nc.dram_tensor("bu", [128, NDT], F32, kind="ExternalInput").ap(),
            "gv": nc.dram_tensor("gv", [128, NTT], F32, kind="ExternalInput").ap(),
            "y": nc.dram_tensor("y", [KTOK, H], F32, kind="ExternalOutput").ap(),
        }
        with tile.TileContext(nc) as tc, ExitStack() as ctx:
            _ffn_body(ctx, tc, **aps)
        nc.compile()
        _NC_CACHE["nc"] = nc
    return _NC_CACHE["nc"]


# ---------------------------------------------------------------- host side
_CPU = jax.devices("cpu")[0]


def _route(xf, router_w, router_b):
    """Exact replica of the reference routing, eagerly on CPU so the top-k
    index selection is bit-identical to the reference."""
    with jax.default_device(_CPU):
        xf = jax.device_put(jnp.asarray(xf), _CPU)
        clean_h = xf @ jax.device_put(jnp.asarray(router_w), _CPU) + jax.device_put(
            jnp.asarray(router_b), _CPU
        )
        u = jax.random.uniform(jax.random.key(42), clean_h.shape, dtype=clean_h.dtype)
        gumbel = -jnp.log(-jnp.log(u + EPS) + EPS)
        h = clean_h + gumbel  # ROUTER_TEMP = 1.0
        Sm = jax.nn.softmax(h, axis=-1)
        Gt, idxt = jax.lax.top_k(Sm.T, KTOK)  # [E, k]
        return np.asarray(Gt), np.asarray(idxt)


def _pack_weights_np(w_gate, w_up, w_down, b_gate, b_up, b_down):
    bf16 = jnp.bfloat16.dtype

    wg_p = np.zeros((H, DFFP), np.float32)
    wg_p[:, :DFF] = w_gate
    wu_p = np.zeros((H, DFFP), np.float32)
    wu_p[:, :DFF] = w_up
    wd_p = np.zeros((DFFP, H), np.float32)
    wd_p[:DFF, :] = w_down
    wd_p[DFF, :] = b_down  # ones-row in h makes the matmul add b_down

    # [H, DFFP] -> [p, ti, kc*128+m]
    wg_dev = np.ascontiguousarray(
        wg_p.reshape(NKC, 128, NDT, 128).transpose(1, 2, 0, 3).reshape(128, NDT, H)
    ).astype(bf16)
    wu_dev = np.ascontiguousarray(
        wu_p.reshape(NKC, 128, NDT, 128).transpose(1, 2, 0, 3).reshape(128, NDT, H)
    ).astype(bf16)
    # [DFFP, H] -> [p, hc, dc*512+m]
    wd_dev = np.ascontiguousarray(
        wd_p.reshape(NDT, 128, NHC, 512).transpose(1, 2, 0, 3).reshape(128, NHC, NDT * 512)
    ).astype(bf16)

    bgp = np.zeros((DFFP,), np.float32)
    bgp[:DFF] = b_gate
    bup = np.zeros((DFFP,), np.float32)
    bup[:DFF] = b_up
    # pad slot DFF: silu(16)*0.0625 == 1.0 exactly in bf16 -> ones-row in h
    # whose contraction against wd's b_down row adds the bias for free.
    bgp[DFF] = 16.0
    bup[DFF] = 0.0625
    bg_dev = np.ascontiguousarray(bgp.reshape(NDT, 128).T)
    bu_dev = np.ascontiguousarray(bup.reshape(NDT, 128).T)

    return wg_dev, wu_dev, wd_dev, bg_dev, bu_dev


def _gather_pack_fn(xf, idxt):
    # toks[e] = xf[idxt[e]]  -> xt[e][p, kc*1024+t] = toks[e][t, kc*128+p]
    toks = jnp.take(xf, idxt.reshape(-1), axis=0).reshape(E, KTOK, H)
    xt = toks.reshape(E, KTOK, NKC, 128).transpose(0, 3, 2, 1).reshape(E, 128, NKC * KTOK)
    return xt.astype(jnp.bfloat16)


def _scatter_out_fn(dev_outs, idxt):
    # dev_outs: [E, KTOK, H] (already G-weighted, b_down included)
    yf = jnp.zeros((NTOK, H), jnp.float32)
    yf = yf.at[idxt.reshape(-1)].add(dev_outs.reshape(-1, H))
    return yf


_gather_pack = jax.jit(_gather_pack_fn)
_scatter_out = jax.jit(_scatter_out_fn)

LAST_RESULTS = None


def kernel(x, router_w, router_b, w_gate, b_gate, w_up, b_up, w_down, b_down):
    global LAST_RESULTS
    B, S, _ = x.shape
    xf_np = np.asarray(x, dtype=np.float32).reshape(NTOK, H)

    Gt, idxt = _route(xf_np, np.asarray(router_w), np.asarray(router_b))

    with jax.default_device(_CPU):
        xt_all = np.asarray(
            _gather_pack(jax.device_put(xf_np, _CPU), jax.device_put(idxt, _CPU))
        )

    wg_dev, wu_dev, wd_dev, bg_dev, bu_dev = _pack_weights_np(
        np.asarray(w_gate, np.float32),
        np.asarray(w_up, np.float32),
        np.asarray(w_down, np.float32),
        np.asarray(b_gate, np.float32),
        np.asarray(b_up, np.float32),
        np.asarray(b_down, np.float32),
    )

    # G columns per expert: gv[p, tt] = G[tt*128+p] for expert e; Gt is [E, k]
    gv_all = np.ascontiguousarray(
        Gt.reshape(E, NTT, 128).transpose(0, 2, 1).astype(np.float32)
    )

    nc = _get_program()
    in_maps = []
    for e in range(E):
        in_maps.append(
            {
                "xt": np.ascontiguousarray(xt_all[e]),
                "wg": wg_dev,
                "wu": wu_dev,
                "wd": wd_dev,
                "bg": bg_dev,
                "bu": bu_dev,
                "gv": gv_all[e],
            }
        )
    res = run_bass_kernel_spmd(nc, in_maps, list(range(E)))
    LAST_RESULTS = res

    dev_outs = np.stack([res.results[e]["y"] for e in range(E)], axis=0)

    with jax.default_device(_CPU):
        yf = np.asarray(
            _scatter_out(jax.device_put(dev_outs, _CPU), jax.device_put(idxt, _CPU))
        )
    return yf.reshape(B, S, H)
